# revision 3
# baseline (speedup 1.0000x reference)
"""Trainium2 Bass kernel for a tanh-RNN language model.

Model (per reference):
    emb    = wte[x]                                   [B, T, E]
    h_t    = tanh(concat([emb_t, h_{t-1}]) @ Cw_w.T + Cw_b)   (scan over T)
    logits = hidden @ head_w.T + head_b               [B, T, V]

Shapes: B=256, T=1024, E=64, H=256, V=256 (fp32 inputs/outputs).

Strategy (8 NeuronCores, data-parallel over batch; 32 sequences/core):
  * Host: embedding gather + transpose to embT [E+1, T*32] (last row = 1.0
    so the Cw bias rides the emb matmul), weights pre-transposed, all bf16
    (validated: 0.36% rel err end-to-end vs fp32 reference).
  * Device, per chunk of 16 timesteps:
      - Z = We_aug.T.T @ embT_chunk accumulated into 2 PSUM banks
        (m-halves of H), start=True.
      - per step: 4 matmuls (Wh.T quadrants over k-half x m-half) accumulate
        into the 32-column slice of those banks (start=False), then tanh on
        ScalarE per m-half -> hs chunk [128, 2, 512] bf16.  The m-half split
        lets ACT(half0) overlap PE matmuls of half1.
      - head: logits rows computed with stationary = hs column blocks,
        moving = head_w.T [128, 256]; + bias on VectorE; DMA out in native
        [b, t, v] layout (1KB contiguous runs).
"""

import numpy as np
import ml_dtypes

B, T, E, H, V = 256, 1024, 64, 256, 256
NCORES = 8
BS = B // NCORES          # 32 sequences per core
TC = 16                   # timesteps per chunk (one PSUM bank pair)
NCHUNK = T // TC          # 64
SC = BS * TC              # 512 columns per chunk

BF = ml_dtypes.bfloat16

_cache = {}


def _build_bass():
    import concourse.mybir as mybir
    import concourse.tile as tile
    from concourse import bacc
    from contextlib import ExitStack

    f32 = mybir.dt.float32
    bf16 = mybir.dt.bfloat16
    TANH = mybir.ActivationFunctionType.Tanh

    nc = bacc.Bacc("TRN2", target_bir_lowering=False, debug=False)

    embT_d = nc.dram_tensor("embT", [E + 1, T * BS], bf16, kind="ExternalInput")
    h0_d = nc.dram_tensor("h0", [128, 2, BS], bf16, kind="ExternalInput")
    whT_d = nc.dram_tensor("whT", [2, 2, 128, 128], bf16, kind="ExternalInput")
    weT_d = nc.dram_tensor("weT", [2, E + 1, 128], bf16, kind="ExternalInput")
    hwT_d = nc.dram_tensor("hwT", [2, 128, V], bf16, kind="ExternalInput")
    hb_d = nc.dram_tensor("hb", [128, V], f32, kind="ExternalInput")
    out_d = nc.dram_tensor("out", [BS, T, V], f32, kind="ExternalOutput")

    embT_a, h0_a, whT_a = embT_d.ap(), h0_d.ap(), whT_d.ap()
    weT_a, hwT_a, hb_a, out_a = weT_d.ap(), hwT_d.ap(), hb_d.ap(), out_d.ap()

    with tile.TileContext(nc) as tc, ExitStack() as ctx:
        const = ctx.enter_context(tc.tile_pool(name="const", bufs=1))
        emb_pool = ctx.enter_context(tc.tile_pool(name="emb_pool", bufs=3))
        hs_pool = ctx.enter_context(tc.tile_pool(name="hs_pool", bufs=3))
        osb_pool = ctx.enter_context(tc.tile_pool(name="osb_pool", bufs=2))
        pre_pool = ctx.enter_context(tc.tile_pool(name="pre_pool", bufs=2, space="PSUM"))
        hps_pool = ctx.enter_context(tc.tile_pool(name="hps_pool", bufs=4, space="PSUM"))

        # ---- constants into SBUF ----
        wh = {}
        for k in range(2):
            for m in range(2):
                t_ = const.tile([128, 128], bf16, name=f"wh_{k}{m}")
                nc.sync.dma_start(t_[:], whT_a[k, m])
                wh[(k, m)] = t_
        we = []
        for m in range(2):
            t_ = const.tile([E + 1, 128], bf16, name=f"we_{m}")
            nc.sync.dma_start(t_[:], weT_a[m])
            we.append(t_)
        hw = []
        for k in range(2):
            t_ = const.tile([128, V], bf16, name=f"hw_{k}")
            nc.sync.dma_start(t_[:], hwT_a[k])
            hw.append(t_)
        hb_t = const.tile([128, V], f32, name="hb_t")
        nc.sync.dma_start(hb_t[:], hb_a[:, :])
        h0_t = const.tile([128, 2, BS], bf16, name="h0_t")
        nc.sync.dma_start(h0_t[:], h0_a[:, :, :])

        hs_prev = None
        hs_list = {}

        def emit_head(c):
            """Head matmuls + bias + DMA for chunk c (4 row-groups)."""
            hs_c = hs_list.pop(c)
            out_sb = osb_pool.tile([128, 4 * V], f32, name="out_sb", tag="out_sb")
            for g in range(4):
                hpsum = hps_pool.tile([128, V], f32, name="hpsum", tag="hpsum")
                nc.tensor.matmul(
                    hpsum[:], hs_c[:, 0, 128 * g : 128 * (g + 1)], hw[0][:],
                    start=True, stop=False,
                )
                nc.tensor.matmul(
                    hpsum[:], hs_c[:, 1, 128 * g : 128 * (g + 1)], hw[1][:],
                    start=False, stop=True,
                )
                nc.vector.tensor_add(
                    out_sb[:, V * g : V * (g + 1)], hpsum[:], hb_t[:]
                )
            for g in range(4):
                dst = out_a[:, TC * c + 4 * g : TC * c + 4 * (g + 1), :].rearrange(
                    "b tl v -> tl b v"
                )
                nc.sync.dma_start(dst, out_sb[:, V * g : V * (g + 1)])

        for c in range(NCHUNK):
            embc = emb_pool.tile([E + 1, SC], bf16, name="embc", tag="embc")
            nc.sync.dma_start(embc[:], embT_a[:, SC * c : SC * (c + 1)])

            pre = pre_pool.tile([128, 2 * SC], f32, name="pre", tag="pre")
            # Z (+bias): emb contribution for all 16 steps, one MM per m-half
            nc.tensor.matmul(pre[:, 0:SC], we[0][:], embc[:], start=True, stop=False,
                             skip_group_check=True)
            nc.tensor.matmul(pre[:, SC : 2 * SC], we[1][:], embc[:], start=True, stop=False,
                             skip_group_check=True)

            hs_c = hs_pool.tile([128, 2, SC], bf16, name="hs_c", tag="hs")
            hs_list[c] = hs_c

            for t in range(TC):
                if t == 0:
                    hp_k0 = h0_t[:, 0, :] if c == 0 else hs_prev[:, 0, BS * (TC - 1) : BS * TC]
                    hp_k1 = h0_t[:, 1, :] if c == 0 else hs_prev[:, 1, BS * (TC - 1) : BS * TC]
                else:
                    hp_k0 = hs_c[:, 0, BS * (t - 1) : BS * t]
                    hp_k1 = hs_c[:, 1, BS * (t - 1) : BS * t]
                s0 = slice(BS * t, BS * (t + 1))
                s1 = slice(SC + BS * t, SC + BS * (t + 1))
                last = t == TC - 1  # 'stop' flags are tracked per PSUM bank
                # m-half 0
                nc.tensor.matmul(pre[:, s0], wh[(0, 0)][:], hp_k0, start=False, stop=False,
                                 skip_group_check=True)
                nc.tensor.matmul(pre[:, s0], wh[(1, 0)][:], hp_k1, start=False, stop=last,
                                 skip_group_check=True)
                nc.scalar.activation(hs_c[:, 0, s0], pre[:, s0], TANH)
                # m-half 1
                nc.tensor.matmul(pre[:, s1], wh[(0, 1)][:], hp_k0, start=False, stop=False,
                                 skip_group_check=True)
                nc.tensor.matmul(pre[:, s1], wh[(1, 1)][:], hp_k1, start=False, stop=last,
                                 skip_group_check=True)
                nc.scalar.activation(hs_c[:, 1, s0], pre[:, s1], TANH)

                # interleave previous chunk's head work into this chunk
                if t == 7 and c > 0:
                    emit_head(c - 1)
            hs_prev = hs_c

        emit_head(NCHUNK - 1)

    nc.compile()
    return nc


def _prep_inputs(x, start, wte, Cw_w, Cw_b, head_w, head_b):
    """Host-side: gather + transpose + cast; returns per-core input dicts."""
    x = np.asarray(x)
    start = np.asarray(start, dtype=np.float32)
    wte = np.asarray(wte, dtype=np.float32)
    Cw_w = np.asarray(Cw_w, dtype=np.float32)
    Cw_b = np.asarray(Cw_b, dtype=np.float32)
    head_w = np.asarray(head_w, dtype=np.float32)
    head_b = np.asarray(head_b, dtype=np.float32)

    # weights (shared by all cores)
    WhT = np.ascontiguousarray(Cw_w[:, E:].T)            # [H_in, H_out]
    whT = np.empty((2, 2, 128, 128), dtype=BF)
    for k in range(2):
        for m in range(2):
            whT[k, m] = Cw_w[:, E:].T[128 * k : 128 * (k + 1), 128 * m : 128 * (m + 1)]
    We_aug = np.concatenate([Cw_w[:, :E], Cw_b[:, None]], axis=1)  # [H, E+1]
    weT = np.empty((2, E + 1, 128), dtype=BF)
    for m in range(2):
        weT[m] = We_aug.T[:, 128 * m : 128 * (m + 1)]
    hwT = np.empty((2, 128, V), dtype=BF)
    for k in range(2):
        hwT[k] = head_w.T[128 * k : 128 * (k + 1), :]
    hb = np.ascontiguousarray(np.broadcast_to(head_b, (128, V))).astype(np.float32)

    h0f = np.broadcast_to(start[0].reshape(2, 128, 1), (2, 128, BS))
    h0 = np.ascontiguousarray(h0f.transpose(1, 0, 2)).astype(BF)  # [128, 2, BS]

    emb = wte[x.astype(np.int64)]          # [B, T, E] fp32
    in_maps = []
    for ci in range(NCORES):
        sh = emb[BS * ci : BS * (ci + 1)]          # [BS, T, E]
        embT = sh.transpose(2, 1, 0).reshape(E, T * BS)  # col = t*BS + b
        embT_aug = np.concatenate(
            [embT, np.ones((1, T * BS), np.float32)], axis=0
        ).astype(BF)
        in_maps.append(
            {
                "embT": embT_aug,
                "h0": h0,
                "whT": whT,
                "weT": weT,
                "hwT": hwT,
                "hb": hb,
            }
        )
    return in_maps


def _get_nc():
    if "nc" not in _cache:
        _cache["nc"] = _build_bass()
    return _cache["nc"]


def run_on_hw(in_maps, trace=False):
    from concourse import bass_utils

    nc = _get_nc()
    res = bass_utils.run_bass_kernel_spmd(
        nc, in_maps, core_ids=list(range(NCORES)), trace=trace
    )
    return res


def kernel(x, start, wte, Cw_w, Cw_b, head_w, head_b):
    in_maps = _prep_inputs(x, start, wte, Cw_w, Cw_b, head_w, head_b)
    res = run_on_hw(in_maps, trace=False)
    out = np.concatenate([r["out"] for r in res.results], axis=0)
    return out.astype(np.float32)


# revision 4
# speedup vs baseline: 1.2816x; 1.2816x over previous
"""Trainium2 Bass kernel for a tanh-RNN language model.

Model (per reference):
    emb    = wte[x]                                   [B, T, E]
    h_t    = tanh(concat([emb_t, h_{t-1}]) @ Cw_w.T + Cw_b)   (scan over T)
    logits = hidden @ head_w.T + head_b               [B, T, V]

Shapes: B=256, T=1024, E=64, H=256, V=256 (fp32 inputs/outputs).

Strategy (8 NeuronCores, data-parallel over batch; 32 sequences/core):
  * Host: embedding gather + transpose to embT [E+1, T*32] (last row = 1.0
    so the Cw bias rides the emb matmul), weights pre-transposed, all bf16
    (validated: 0.36% rel err end-to-end vs fp32 reference).
  * Device, per chunk of 16 timesteps:
      - Z = We_aug.T.T @ embT_chunk accumulated into 2 PSUM banks
        (m-halves of H), start=True.
      - per step: 4 matmuls (Wh.T quadrants over k-half x m-half) accumulate
        into the 32-column slice of those banks (start=False), then tanh on
        ScalarE per m-half -> hs chunk [128, 2, 512] bf16.  The m-half split
        lets ACT(half0) overlap PE matmuls of half1.
      - head: logits rows computed with stationary = hs column blocks,
        moving = head_w.T [128, 256]; + bias on VectorE; DMA out in native
        [b, t, v] layout (1KB contiguous runs).
"""

import numpy as np
import ml_dtypes

B, T, E, H, V = 256, 1024, 64, 256, 256
NCORES = 8
BS = B // NCORES          # 32 sequences per core
TC = 16                   # timesteps per chunk (one PSUM bank pair)
NCHUNK = T // TC          # 64
SC = BS * TC              # 512 columns per chunk

BF = ml_dtypes.bfloat16

_cache = {}


def _build_bass():
    import concourse.mybir as mybir
    import concourse.tile as tile
    from concourse import bacc
    from contextlib import ExitStack

    f32 = mybir.dt.float32
    bf16 = mybir.dt.bfloat16
    TANH = mybir.ActivationFunctionType.Tanh

    nc = bacc.Bacc("TRN2", target_bir_lowering=False, debug=False)

    embT_d = nc.dram_tensor("embT", [E + 1, T * BS], bf16, kind="ExternalInput")
    h0_d = nc.dram_tensor("h0", [128, 2, BS], bf16, kind="ExternalInput")
    whT_d = nc.dram_tensor("whT", [2, 2, 128, 128], bf16, kind="ExternalInput")
    weT_d = nc.dram_tensor("weT", [2, E + 1, 128], bf16, kind="ExternalInput")
    hwT_d = nc.dram_tensor("hwT", [2, 128, V], bf16, kind="ExternalInput")
    hb_d = nc.dram_tensor("hb", [128, V], f32, kind="ExternalInput")
    out_d = nc.dram_tensor("out", [BS, T, V], f32, kind="ExternalOutput")

    embT_a, h0_a, whT_a = embT_d.ap(), h0_d.ap(), whT_d.ap()
    weT_a, hwT_a, hb_a, out_a = weT_d.ap(), hwT_d.ap(), hb_d.ap(), out_d.ap()

    with tile.TileContext(nc) as tc, ExitStack() as ctx:
        const = ctx.enter_context(tc.tile_pool(name="const", bufs=1))
        emb_pool = ctx.enter_context(tc.tile_pool(name="emb_pool", bufs=3))
        hs_pool = ctx.enter_context(tc.tile_pool(name="hs_pool", bufs=3))
        osb_pool = ctx.enter_context(tc.tile_pool(name="osb_pool", bufs=2))
        pre_pool = ctx.enter_context(tc.tile_pool(name="pre_pool", bufs=2, space="PSUM"))
        hps_pool = ctx.enter_context(tc.tile_pool(name="hps_pool", bufs=4, space="PSUM"))

        # ---- constants into SBUF ----
        wh = {}
        for k in range(2):
            for m in range(2):
                t_ = const.tile([128, 128], bf16, name=f"wh_{k}{m}")
                nc.sync.dma_start(t_[:], whT_a[k, m])
                wh[(k, m)] = t_
        we = []
        for m in range(2):
            t_ = const.tile([E + 1, 128], bf16, name=f"we_{m}")
            nc.sync.dma_start(t_[:], weT_a[m])
            we.append(t_)
        hw = []
        for k in range(2):
            t_ = const.tile([128, V], bf16, name=f"hw_{k}")
            nc.sync.dma_start(t_[:], hwT_a[k])
            hw.append(t_)
        hb_t = const.tile([128, V], f32, name="hb_t")
        nc.sync.dma_start(hb_t[:], hb_a[:, :])
        h0_t = const.tile([128, 2, BS], bf16, name="h0_t")
        nc.sync.dma_start(h0_t[:], h0_a[:, :, :])

        hs_prev = None
        hs_list = {}

        def emit_head(c):
            """Head matmuls + bias + DMA for chunk c (4 row-groups)."""
            hs_c = hs_list.pop(c)
            out_sb = osb_pool.tile([128, 4 * V], f32, name="out_sb", tag="out_sb")
            for g in range(4):
                hpsum = hps_pool.tile([128, V], f32, name="hpsum", tag="hpsum")
                nc.tensor.matmul(
                    hpsum[:], hs_c[:, 0, 128 * g : 128 * (g + 1)], hw[0][:],
                    start=True, stop=False,
                )
                nc.tensor.matmul(
                    hpsum[:], hs_c[:, 1, 128 * g : 128 * (g + 1)], hw[1][:],
                    start=False, stop=True,
                )
                nc.vector.tensor_add(
                    out_sb[:, V * g : V * (g + 1)], hpsum[:], hb_t[:]
                )
            for g in range(4):
                dst = out_a[:, TC * c + 4 * g : TC * c + 4 * (g + 1), :].rearrange(
                    "b tl v -> tl b v"
                )
                nc.sync.dma_start(dst, out_sb[:, V * g : V * (g + 1)])

        for c in range(NCHUNK):
            embc = emb_pool.tile([E + 1, SC], bf16, name="embc", tag="embc")
            nc.sync.dma_start(embc[:], embT_a[:, SC * c : SC * (c + 1)])

            pre0 = pre_pool.tile([128, SC], f32, name="pre0", tag="pre0")
            pre1 = pre_pool.tile([128, SC], f32, name="pre1", tag="pre1")
            # Z (+bias): emb contribution for all 16 steps, one MM per m-half
            nc.tensor.matmul(pre0[:, 0:SC], we[0][:], embc[:], start=True, stop=False,
                             skip_group_check=True)
            nc.tensor.matmul(pre1[:, 0:SC], we[1][:], embc[:], start=True, stop=False,
                             skip_group_check=True)

            hs_c = hs_pool.tile([128, 2, SC], bf16, name="hs_c", tag="hs")
            hs_list[c] = hs_c

            for t in range(TC):
                if t == 0:
                    hp_k0 = h0_t[:, 0, :] if c == 0 else hs_prev[:, 0, BS * (TC - 1) : BS * TC]
                    hp_k1 = h0_t[:, 1, :] if c == 0 else hs_prev[:, 1, BS * (TC - 1) : BS * TC]
                else:
                    hp_k0 = hs_c[:, 0, BS * (t - 1) : BS * t]
                    hp_k1 = hs_c[:, 1, BS * (t - 1) : BS * t]
                s0 = slice(BS * t, BS * (t + 1))
                last = t == TC - 1  # 'stop' flags are tracked per PSUM bank
                # m-half 0
                nc.tensor.matmul(pre0[:, s0], wh[(0, 0)][:], hp_k0, start=False, stop=False,
                                 skip_group_check=True)
                nc.tensor.matmul(pre0[:, s0], wh[(1, 0)][:], hp_k1, start=False, stop=last,
                                 skip_group_check=True)
                nc.scalar.activation(hs_c[:, 0, s0], pre0[:, s0], TANH)
                # m-half 1
                nc.tensor.matmul(pre1[:, s0], wh[(0, 1)][:], hp_k0, start=False, stop=False,
                                 skip_group_check=True)
                nc.tensor.matmul(pre1[:, s0], wh[(1, 1)][:], hp_k1, start=False, stop=last,
                                 skip_group_check=True)
                nc.scalar.activation(hs_c[:, 1, s0], pre1[:, s0], TANH)

                # interleave previous chunk's head work into this chunk
                if t == 7 and c > 0:
                    emit_head(c - 1)
            hs_prev = hs_c

        emit_head(NCHUNK - 1)

    nc.compile()
    return nc


def _prep_inputs(x, start, wte, Cw_w, Cw_b, head_w, head_b):
    """Host-side: gather + transpose + cast; returns per-core input dicts."""
    x = np.asarray(x)
    start = np.asarray(start, dtype=np.float32)
    wte = np.asarray(wte, dtype=np.float32)
    Cw_w = np.asarray(Cw_w, dtype=np.float32)
    Cw_b = np.asarray(Cw_b, dtype=np.float32)
    head_w = np.asarray(head_w, dtype=np.float32)
    head_b = np.asarray(head_b, dtype=np.float32)

    # weights (shared by all cores)
    WhT = np.ascontiguousarray(Cw_w[:, E:].T)            # [H_in, H_out]
    whT = np.empty((2, 2, 128, 128), dtype=BF)
    for k in range(2):
        for m in range(2):
            whT[k, m] = Cw_w[:, E:].T[128 * k : 128 * (k + 1), 128 * m : 128 * (m + 1)]
    We_aug = np.concatenate([Cw_w[:, :E], Cw_b[:, None]], axis=1)  # [H, E+1]
    weT = np.empty((2, E + 1, 128), dtype=BF)
    for m in range(2):
        weT[m] = We_aug.T[:, 128 * m : 128 * (m + 1)]
    hwT = np.empty((2, 128, V), dtype=BF)
    for k in range(2):
        hwT[k] = head_w.T[128 * k : 128 * (k + 1), :]
    hb = np.ascontiguousarray(np.broadcast_to(head_b, (128, V))).astype(np.float32)

    h0f = np.broadcast_to(start[0].reshape(2, 128, 1), (2, 128, BS))
    h0 = np.ascontiguousarray(h0f.transpose(1, 0, 2)).astype(BF)  # [128, 2, BS]

    emb = wte[x.astype(np.int64)]          # [B, T, E] fp32
    in_maps = []
    for ci in range(NCORES):
        sh = emb[BS * ci : BS * (ci + 1)]          # [BS, T, E]
        embT = sh.transpose(2, 1, 0).reshape(E, T * BS)  # col = t*BS + b
        embT_aug = np.concatenate(
            [embT, np.ones((1, T * BS), np.float32)], axis=0
        ).astype(BF)
        in_maps.append(
            {
                "embT": embT_aug,
                "h0": h0,
                "whT": whT,
                "weT": weT,
                "hwT": hwT,
                "hb": hb,
            }
        )
    return in_maps


def _get_nc():
    if "nc" not in _cache:
        _cache["nc"] = _build_bass()
    return _cache["nc"]


def run_on_hw(in_maps, trace=False):
    from concourse import bass_utils

    nc = _get_nc()
    res = bass_utils.run_bass_kernel_spmd(
        nc, in_maps, core_ids=list(range(NCORES)), trace=trace
    )
    return res


def kernel(x, start, wte, Cw_w, Cw_b, head_w, head_b):
    in_maps = _prep_inputs(x, start, wte, Cw_w, Cw_b, head_w, head_b)
    res = run_on_hw(in_maps, trace=False)
    out = np.concatenate([r["out"] for r in res.results], axis=0)
    return out.astype(np.float32)


# revision 5
# speedup vs baseline: 1.6368x; 1.2772x over previous
"""Trainium2 Bass kernel for a tanh-RNN language model.

Model (per reference):
    emb    = wte[x]                                   [B, T, E]
    h_t    = tanh(concat([emb_t, h_{t-1}]) @ Cw_w.T + Cw_b)   (scan over T)
    logits = hidden @ head_w.T + head_b               [B, T, V]

Shapes: B=256, T=1024, E=64, H=256, V=256 (fp32 in/out).

Strategy (8 NeuronCores, data-parallel over batch; 32 sequences/core):
  * Host: embedding gather + transpose to embT (last row = 1.0 so the Cw
    bias rides the emb matmul), weights pre-transposed, all bf16
    (validated: ~0.4% rel err end-to-end vs the fp32 reference).
  * The recurrence is latency-bound (PE->tanh->PE round trip), so each
    core runs TWO independent 16-sequence groups, software-pipelined:
    while group A's tanh (ScalarE) is in flight, group B's matmuls issue
    on the PE, and vice versa.
  * Per chunk of 32 timesteps, per group: Z = We_aug.T.T @ embT_chunk
    lands in a [128, 2x512] PSUM pair (start=True); each step's 4
    quadrant matmuls of Wh.T accumulate into 16-column slices
    (start=False), then one tanh over both banks (2D AP) writes the
    bf16 hidden-state chunk hs[g] [128, 2, 512].
  * Head: logits rows = (hs column block as stationary) @ head_w.T,
    + bias on VectorE, DMA'd out in native [b, t, v] layout (1KB runs).
"""

import numpy as np
import ml_dtypes

B, T, E, H, V = 256, 1024, 64, 256, 256
NCORES = 8
BS = B // NCORES          # 32 sequences per core
G = 2                     # pipelined groups per core
GB = BS // G              # 16 sequences per group
TC = 32                   # timesteps per chunk (fills one PSUM bank per m-half)
NCHUNK = T // TC          # 32
SCG = GB * TC             # 512 columns per (chunk, group)

BF = ml_dtypes.bfloat16

_cache = {}


def _build_bass():
    import concourse.mybir as mybir
    import concourse.tile as tile
    from concourse import bacc
    from contextlib import ExitStack

    f32 = mybir.dt.float32
    bf16 = mybir.dt.bfloat16
    TANH = mybir.ActivationFunctionType.Tanh

    nc = bacc.Bacc("TRN2", target_bir_lowering=False, debug=False)

    embT_d = nc.dram_tensor("embT", [E + 1, T * BS], bf16, kind="ExternalInput")
    h0_d = nc.dram_tensor("h0", [128, 2, BS], bf16, kind="ExternalInput")
    whT_d = nc.dram_tensor("whT", [2, 2, 128, 128], bf16, kind="ExternalInput")
    weT_d = nc.dram_tensor("weT", [2, E + 1, 128], bf16, kind="ExternalInput")
    hwT_d = nc.dram_tensor("hwT", [2, 128, V], bf16, kind="ExternalInput")
    hb_d = nc.dram_tensor("hb", [128, V], f32, kind="ExternalInput")
    out_d = nc.dram_tensor("out", [BS, T, V], f32, kind="ExternalOutput")

    embT_a, h0_a, whT_a = embT_d.ap(), h0_d.ap(), whT_d.ap()
    weT_a, hwT_a, hb_a, out_a = weT_d.ap(), hwT_d.ap(), hb_d.ap(), out_d.ap()

    with tile.TileContext(nc) as tc, ExitStack() as ctx:
        const = ctx.enter_context(tc.tile_pool(name="const", bufs=1))
        emb_pool = ctx.enter_context(tc.tile_pool(name="emb_pool", bufs=3))
        hs_pool = ctx.enter_context(tc.tile_pool(name="hs_pool", bufs=3))
        osb_pool = ctx.enter_context(tc.tile_pool(name="osb_pool", bufs=4))
        pre_pool = ctx.enter_context(tc.tile_pool(name="pre_pool", bufs=1, space="PSUM"))
        hps_pool = ctx.enter_context(tc.tile_pool(name="hps_pool", bufs=4, space="PSUM"))

        # ---- constants into SBUF ----
        wh = {}
        for k in range(2):
            for m in range(2):
                t_ = const.tile([128, 128], bf16, name=f"wh_{k}{m}")
                nc.sync.dma_start(t_[:], whT_a[k, m])
                wh[(k, m)] = t_
        we = []
        for m in range(2):
            t_ = const.tile([E + 1, 128], bf16, name=f"we_{m}")
            nc.sync.dma_start(t_[:], weT_a[m])
            we.append(t_)
        hw = []
        for k in range(2):
            t_ = const.tile([128, V], bf16, name=f"hw_{k}")
            nc.sync.dma_start(t_[:], hwT_a[k])
            hw.append(t_)
        hb_t = const.tile([128, V], f32, name="hb_t")
        nc.sync.dma_start(hb_t[:], hb_a[:, :])
        h0_t = const.tile([128, 2, BS], bf16, name="h0_t")
        nc.sync.dma_start(h0_t[:], h0_a[:, :, :])

        hs_list = {}

        def emit_head_unit(c, g, u):
            """Head for row-group u (8 steps x 16 seqs) of chunk c, group g."""
            hs_c = hs_list[(c, g)]
            out_sb = osb_pool.tile([128, V], f32, name="out_sb", tag="out_sb")
            hpsum = hps_pool.tile([128, V], f32, name="hpsum", tag="hpsum")
            nc.tensor.matmul(
                hpsum[:], hs_c[:, 0, 128 * u : 128 * (u + 1)], hw[0][:],
                start=True, stop=False,
            )
            nc.tensor.matmul(
                hpsum[:], hs_c[:, 1, 128 * u : 128 * (u + 1)], hw[1][:],
                start=False, stop=True,
            )
            nc.vector.tensor_add(out_sb[:, :], hpsum[:], hb_t[:])
            t0 = TC * c + 8 * u
            dst = out_a[GB * g : GB * (g + 1), t0 : t0 + 8, :].rearrange(
                "b tl v -> tl b v"
            )
            nc.sync.dma_start(dst, out_sb[:, :])

        for c in range(NCHUNK):
            embc = emb_pool.tile([E + 1, G * SCG], bf16, name="embc", tag="embc")
            nc.sync.dma_start(embc[:], embT_a[:, G * SCG * c : G * SCG * (c + 1)])

            pre = []
            for g in range(G):
                p = pre_pool.tile([128, 2 * SCG], f32, name=f"pre{g}", tag=f"pre{g}")
                pre.append(p)
                for m in range(2):
                    nc.tensor.matmul(
                        p[:, SCG * m : SCG * (m + 1)], we[m][:],
                        embc[:, SCG * g : SCG * (g + 1)],
                        start=True, stop=False, skip_group_check=True,
                    )
                hs_list[(c, g)] = hs_pool.tile(
                    [128, 2, SCG], bf16, name=f"hs{g}", tag=f"hs{g}"
                )

            # head work for the previous chunk, interleaved (8 units / 32 steps)
            heads = [(c - 1, gg, uu) for uu in range(4) for gg in range(G)] if c > 0 else []

            for t in range(TC):
                s = slice(GB * t, GB * (t + 1))
                last = t == TC - 1
                for g in range(G):
                    p = pre[g]
                    hs_c = hs_list[(c, g)]
                    if t == 0:
                        hp = h0_t[:, :, GB * g : GB * (g + 1)] if c == 0 else \
                            hs_list[(c - 1, g)][:, :, GB * (TC - 1) : GB * TC]
                    else:
                        hp = hs_c[:, :, GB * (t - 1) : GB * t]
                    for m in range(2):
                        sm = slice(SCG * m + GB * t, SCG * m + GB * (t + 1))
                        nc.tensor.matmul(p[:, sm], wh[(0, m)][:], hp[:, 0, :],
                                         start=False, stop=False, skip_group_check=True)
                        nc.tensor.matmul(p[:, sm], wh[(1, m)][:], hp[:, 1, :],
                                         start=False, stop=last, skip_group_check=True)
                    # one tanh over both m-half banks (2D AP), writes bf16 hs
                    pin = p[:, :].rearrange("p (m c) -> p m c", m=2)[:, :, s]
                    nc.scalar.activation(hs_c[:, :, s], pin, TANH)

                if t % 4 == 3 and heads:
                    emit_head_unit(*heads.pop(0))

            if c > 1:
                for g in range(G):
                    hs_list.pop((c - 2, g), None)

        for uu in range(4):
            for gg in range(G):
                emit_head_unit(NCHUNK - 1, gg, uu)

    nc.compile()
    return nc


def _prep_inputs(x, start, wte, Cw_w, Cw_b, head_w, head_b):
    """Host-side: gather + transpose + cast; returns per-core input dicts."""
    x = np.asarray(x)
    start = np.asarray(start, dtype=np.float32)
    wte = np.asarray(wte, dtype=np.float32)
    Cw_w = np.asarray(Cw_w, dtype=np.float32)
    Cw_b = np.asarray(Cw_b, dtype=np.float32)
    head_w = np.asarray(head_w, dtype=np.float32)
    head_b = np.asarray(head_b, dtype=np.float32)

    whT = np.empty((2, 2, 128, 128), dtype=BF)
    WhT_f = Cw_w[:, E:].T
    for k in range(2):
        for m in range(2):
            whT[k, m] = WhT_f[128 * k : 128 * (k + 1), 128 * m : 128 * (m + 1)]
    We_aug = np.concatenate([Cw_w[:, :E], Cw_b[:, None]], axis=1)  # [H, E+1]
    weT = np.empty((2, E + 1, 128), dtype=BF)
    for m in range(2):
        weT[m] = We_aug.T[:, 128 * m : 128 * (m + 1)]
    hwT = np.empty((2, 128, V), dtype=BF)
    for k in range(2):
        hwT[k] = head_w.T[128 * k : 128 * (k + 1), :]
    hb = np.ascontiguousarray(np.broadcast_to(head_b, (128, V))).astype(np.float32)

    h0f = np.broadcast_to(start[0].reshape(2, 128, 1), (2, 128, BS))
    h0 = np.ascontiguousarray(h0f.transpose(1, 0, 2)).astype(BF)  # [128, 2, BS]

    emb = wte[x.astype(np.int64)]          # [B, T, E] fp32
    in_maps = []
    for ci in range(NCORES):
        sh = emb[BS * ci : BS * (ci + 1)]  # [BS, T, E]
        # column layout: col = c*(G*SCG) + g*SCG + tl*GB + bl
        #   where t = c*TC + tl, b = g*GB + bl
        e5 = sh.reshape(G, GB, NCHUNK, TC, E)                  # [g, bl, c, tl, E]
        embT = e5.transpose(4, 2, 0, 3, 1).reshape(E, T * BS)  # [E, (c g tl bl)]
        embT_aug = np.concatenate(
            [embT, np.ones((1, T * BS), np.float32)], axis=0
        ).astype(BF)
        in_maps.append(
            {"embT": embT_aug, "h0": h0, "whT": whT, "weT": weT, "hwT": hwT, "hb": hb}
        )
    return in_maps


def _get_nc():
    if "nc" not in _cache:
        _cache["nc"] = _build_bass()
    return _cache["nc"]


def run_on_hw(in_maps, trace=False):
    from concourse import bass_utils

    nc = _get_nc()
    res = bass_utils.run_bass_kernel_spmd(
        nc, in_maps, core_ids=list(range(NCORES)), trace=trace
    )
    return res


def kernel(x, start, wte, Cw_w, Cw_b, head_w, head_b):
    in_maps = _prep_inputs(x, start, wte, Cw_w, Cw_b, head_w, head_b)
    res = run_on_hw(in_maps, trace=False)
    out = np.concatenate([r["out"] for r in res.results], axis=0)
    return out.astype(np.float32)
